# revision 2
# baseline (speedup 1.0000x reference)
"""Trainium2 Bass kernel for nn_ModelNew_17411797418162.

Computation (per (b,s) sample):
  mixed = h_res @ x            # [4,4] @ [4,1024]
  out   = mixed * h_out[None,:] + h_post[:,None] * x

Sharding: pure data parallel over the leading batch dim B=8 -> 1 batch/core.

Per-core design (memory-bound). The rel-err gate (2e-2) leaves huge
precision headroom over fp32, so all HBM traffic is fp16 (verified rel err
~9e-4 end to end): x in 16MB, out 16MB, h_out plane 4MB, block-diag
weights 2MB, diag(h_post) 2MB = 40MB/core -> ~112us at the 358GB/s
per-core HBM roofline (vs 76MB/270us for the fp32 baseline).

Math restructure to minimize PSUM-evacuation work (ACT/DVE are 1x rate on
PSUM-sourced ops, ~1.2us per [128,1024] op):
  out = (h_res @ (x * bcast(h_out))) + diag(h_post) @ x
- bcast(h_out) onto the 4 stream rows runs on the PE: a constant 0/1
  fp16 matrix E (lhsT, K=32 via tile_position) replicates 32 samples'
  h_out rows to 128 (sample,stream) partitions in PSUM (psh).
- DVE: y = x * psh (one PSUM-sourced tensor_tensor per k, 1.19us).
- PE: ps = W_blk.T @ y + DG_blk.T @ x accumulated in one PSUM tile
  (W = block-diag scatter of h_res^T, DG = diag(h_post); both fp16,
  scattered host-side - layout only).
- ACT: one PSUM-sourced copy evacuates ps -> out fp16 (1.15us).
Engine budget/core: DMA 112us (bound), DVE 76us, ACT 73us, PE ~60us.

Layout: rows = flattened (s, stream), partition p of sub-block blk holds
row 128*blk + p (sample 32*blk + p//4, stream p%4), matching the W/DG
block structure (32 samples x 4 streams = 128 partitions per block).
"""
import numpy as np

import concourse.bacc as bacc
import concourse.tile as tile
import concourse.mybir as mybir
from concourse.bass_utils import run_bass_kernel_spmd

B, S, N, D = 8, 2048, 4, 1024
NCORES = 8
ROWS = S * N              # 8192 flattened rows per core
NSB = 16                  # super-blocks (128 samples / 512 rows) per core
SUBS = 4                  # sub-blocks (32 samples / 128 rows) per super-block
NG = 4                    # super-blocks per h_out load group
F32 = mybir.dt.float32
FP16 = mybir.dt.float16

_cache = {}


def build_program(iters: int = 1, mode: str = "full"):
    """Build the SPMD Bass program (one core's view). Cached per (iters, mode).

    mode: "full" = real kernel; ablations for bottleneck isolation:
      "dma"   = loads + store only (wrong output values)
      "nodve" = skip the y multiply, feed x to both matmuls (wrong values)
    """
    if (iters, mode) in _cache:
        return _cache[(iters, mode)]

    nc = bacc.Bacc("TRN2", target_bir_lowering=False, debug=False)
    x = nc.dram_tensor("x", [ROWS, D], FP16, kind="ExternalInput")
    w = nc.dram_tensor("w", [64, 128, 128], FP16, kind="ExternalInput")
    dg = nc.dram_tensor("dg", [64, 128, 128], FP16, kind="ExternalInput")
    ho = nc.dram_tensor("ho", [S, D], FP16, kind="ExternalInput")
    e4 = nc.dram_tensor("e4", [128, 128], FP16, kind="ExternalInput")
    out = nc.dram_tensor("out", [ROWS, D], FP16, kind="ExternalOutput")

    with tile.TileContext(nc) as tc:
        with (
            tc.tile_pool(name="const", bufs=1) as cpool,
            tc.tile_pool(name="big", bufs=3) as bpool,
            tc.tile_pool(name="hop", bufs=2) as hpool,
            tc.tile_pool(name="mid", bufs=3) as mpool,
            tc.tile_pool(name="psum", bufs=4, space="PSUM") as ppool,
        ):
            e4_t = cpool.tile([128, 128], FP16)
            nc.gpsimd.dma_start(e4_t[:], e4.ap())
            # resident block weights: w_all[r, (b, c)] = w[b, r, c]
            w_all = cpool.tile([128, 64 * 128], FP16)
            nc.gpsimd.dma_start(
                w_all[:].rearrange("r (b c) -> r b c", b=64),
                w.ap().rearrange("b r c -> r b c"))
            dg_all = cpool.tile([128, 64 * 128], FP16)
            nc.gpsimd.dma_start(
                dg_all[:].rearrange("r (b c) -> r b c", b=64),
                dg.ap().rearrange("b r c -> r b c"))

            def body():
                for sb2 in range(NSB // NG):
                    # h_out rows for 4 super-blocks (512 samples), one 1MB
                    # DMA: ho_g[p, (g, d)] = ho[512*sb2 + 128*g + p, d]
                    ho_g = hpool.tile([128, NG * D], FP16, tag="hog")
                    nc.gpsimd.dma_start(
                        ho_g[:].rearrange("p (g d) -> p g d", g=NG),
                        ho.ap()[512 * sb2:512 * (sb2 + 1), :].rearrange(
                            "(g p) d -> p g d", g=NG))

                    for g in range(NG):
                        sb = NG * sb2 + g
                        # x rows 512*sb..512*(sb+1): [p=128, (k, d)]
                        x_t = bpool.tile([128, SUBS * D], FP16, tag="x")
                        src = x.ap()[512 * sb:512 * (sb + 1), :].rearrange(
                            "(k p) d -> p k d", k=SUBS)
                        nc.sync.dma_start(
                            x_t[:].rearrange("p (k d) -> p k d", k=SUBS), src)

                        out_sb = bpool.tile([128, SUBS * D], FP16, tag="out")

                        # broadcast h_out onto stream rows for all 4 k up
                        # front so the PE never head-of-line blocks on DVE
                        pshs = []
                        for k in range(SUBS):
                            psh = ppool.tile([128, D], F32, tag="ps")
                            for c in range(2):
                                nc.tensor.matmul(
                                    psh[:, 512 * c:512 * (c + 1)],
                                    e4_t[32 * k:32 * (k + 1), :],
                                    ho_g[32 * k:32 * (k + 1),
                                         D * g + 512 * c:D * g + 512 * (c + 1)],
                                    start=True, stop=True,
                                    tile_position=(32 * k, 0))
                            pshs.append(psh)

                        for k in range(SUBS):
                            blk = SUBS * sb + k
                            xk = x_t[:, D * k:D * (k + 1)]
                            if mode == "dma":
                                continue
                            # y = x * bcast(h_out)
                            if mode == "full":
                                y_t = mpool.tile([128, D], FP16, tag="y")
                                nc.vector.tensor_mul(y_t[:], xk, pshs[k][:])
                            else:
                                y_t = x_t  # ablation: skip DVE
                            yk = y_t[:] if mode == "full" else xk

                            # ps = W.T @ y + DG.T @ x (fp32 PSUM accum)
                            ps = ppool.tile([128, D], F32, tag="ps")
                            lhsW = w_all[:, 128 * blk:128 * (blk + 1)]
                            lhsD = dg_all[:, 128 * blk:128 * (blk + 1)]
                            for c in range(2):
                                sl = slice(512 * c, 512 * (c + 1))
                                nc.tensor.matmul(
                                    ps[:, sl], lhsW, yk[:, sl],
                                    start=True, stop=False)
                                nc.tensor.matmul(
                                    ps[:, sl], lhsD, xk[:, sl],
                                    start=False, stop=True)
                            # evacuate (converts f32 -> fp16)
                            nc.scalar.copy(out_sb[:, D * k:D * (k + 1)], ps[:])

                        src_sb = x_t if mode == "dma" else out_sb
                        dst = out.ap()[512 * sb:512 * (sb + 1), :].rearrange(
                            "(k p) d -> p k d", k=SUBS)
                        nc.scalar.dma_start(
                            dst, src_sb[:].rearrange("p (k d) -> p k d",
                                                     k=SUBS))

            if iters == 1:
                body()
            else:
                with tc.For_i(0, iters, 1):
                    body()

    nc.compile()
    _cache[(iters, mode)] = nc
    return nc


def make_in_maps(x, h_res, h_out, h_post):
    """Split full inputs into per-core input maps (host-side, layout +
    dtype-conversion only)."""
    x = np.ascontiguousarray(x, dtype=np.float32)
    h_res = np.ascontiguousarray(h_res, dtype=np.float32)
    h_out = np.ascontiguousarray(h_out, dtype=np.float32)
    h_post = np.ascontiguousarray(h_post, dtype=np.float32)

    # stream-replication matrix: e4[32k + q, 4q + i] = 1.0
    e4 = np.zeros((128, 128), np.float16)
    q = np.arange(128)
    for i in range(4):
        e4[q, 4 * (q % 32) + i] = 1.0

    idx = np.arange(32)
    r = np.arange(128)
    in_maps = []
    for c in range(NCORES):
        xc = x[c].reshape(ROWS, D).astype(np.float16)
        # Block-diagonal weights: W[b, 4p+j, 4p+i] = h_res[c, 32b+p, i, j]
        hr = h_res[c].reshape(64, 32, 4, 4)            # [b, p, i, j]
        Wb = np.zeros((64, 32, 4, 32, 4), np.float16)  # [b, (p,j), (p,i)]
        Wb[:, idx, :, idx, :] = hr.transpose(1, 0, 3, 2).astype(np.float16)
        # Diagonal h_post: DG[b, r, r] = h_post_flat[128b + r]
        Dg = np.zeros((64, 128, 128), np.float16)
        Dg[:, r, r] = h_post[c].reshape(64, 128).astype(np.float16)
        in_maps.append({
            "x": xc,
            "w": Wb.reshape(64, 128, 128),
            "dg": Dg,
            "ho": h_out[c].astype(np.float16),
            "e4": e4,
        })
    return in_maps


def kernel(x, h_res, h_out, h_post):
    nc = build_program(iters=1)
    in_maps = make_in_maps(x, h_res, h_out, h_post)
    res = run_bass_kernel_spmd(nc, in_maps, list(range(NCORES)))
    out = np.stack([res.results[c]["out"].astype(np.float32).reshape(S, N, D)
                    for c in range(NCORES)])
    return out
